# revision 2
# baseline (speedup 1.0000x reference)
"""StyleGAN2-style modulated 3x3 conv layer on 8 TRN2 NeuronCores.

Math (per sample b):
    style = latent @ (fc_weight * LAT**-0.5).T + fc_bias            [CIN]
    w     = weight * style[None,:,None,None]                        [COUT,CIN,3,3]
    w     = w * rsqrt(sum(w*w, (1,2,3)) + EPS) * w_mul_conv
    out   = lrelu(conv2d(x, w, pad=1) + bias, 0.2)

Folded form used here (weights stay shared across the batch):
    x_mod[i]  = x[i] * style[i]
    conv      = conv2d(x_mod, weight)             (shared weight, fp16 matmuls)
    d[o]      = w_mul_conv * rsqrt(sum_i style[i]^2 * ssq[o,i] + EPS)
                with ssq[o,i] = sum_k weight[o,i,kh,kw]^2           (host precomp)
    out[o]    = lrelu(conv[o] * d[o] + bias[o], 0.2)

Sharding: data-parallel over batch B=8, one sample per core; conv/FC weights
replicated. The conv uses width-direction Winograd F(4,3): weights are
G-transformed on host into 18 [CIN,COUT] fp16 matrices (3 kh taps x 6 g); the
modulated image ships from host as 4 width-phase planes (plane a holds padded
cols 4j+a, 17 j-entries) so every on-device B^T combo reads packed fp16 and
rides the DVE 2x/4x fast paths; height stays a direct 3-tap accumulation via
row shifts. Each (co-tile, 32-row block) accumulates 6 PSUM banks (one per g,
12 matmuls each); the output transform Y0=m0+p+r / Y1=q+2s / Y2=p+4r /
Y3=q+8s+m5 (p=m1+m2, q=m1-m2, r=m3+m4, s=m3-m4) runs as fp16 tensor-tensor
ops on DVE, and the ACT engine applies lrelu(Y*d_fold + bias) while writing
the 4-way width interleave (ACT is stride-insensitive; DVE is not); each
block's activations+store are emitted one block late so the in-order ACT
queue frees PSUM banks without waiting on DVE.

Software pipelining: the V planes are split into row halves (rows 0..33 feed
the b=0 blocks, rows 32..65 feed b=1), the conv runs all-b0-then-all-b1, and
each half's B^T build is emitted half a rep ahead (next rep's inputs/style/
modulate/demod emit between the halves), so the ~40us DVE V-build hides
completely under PE instead of stalling each rep's start.
295K PE columns vs 393K for width F(2,3) and 608K for direct 3x3.

reps>1 replicates the per-sample body with this pipelining so consecutive
reps run back-to-back - used for slope-timing on HW (per-rep time =
steady-state kernel throughput).
"""

import numpy as np

B, CIN, COUT, K, LAT, H, W = 8, 512, 512, 3, 512, 64, 64
EPS = 1e-8
W_MUL_FC = LAT**-0.5
W_MUL_CONV = (2.0**0.5) * (CIN * K * K) ** -0.5

P = 128
CI_T = CIN // P  # 4 input-channel tiles
CO_T = COUT // P  # 4 output-channel tiles
LA_T = LAT // P  # 4 latent tiles
WP = 72  # padded row width the phases are cut from (1+64+1 real, rest zero)
HP = H + 2  # padded height (66)
G = 0  # no guard columns needed with the phase-plane layout
NJ = W // 4  # 16 winograd tiles per row (F(4,3) along width)
JW = NJ + 1  # 17 j-entries per phase plane (tile j covers cols 4j..4j+5)
PLC = HP * JW  # 1122 cols per phase plane
XCOLS = G + 4 * PLC + G  # 4488
HR = 34  # rows per V half (b=0: rows 0..33, b=1: rows 32..65; 2 overlap)
VHC = HR * NJ  # 544 cols per V half-plane
NB = 2  # row-blocks per (co): 32 rows x 16 = 512 cols = one PSUM bank
NMAX = 512

_COMPILED = {}


def _build_nc(reps=1):
    import concourse.bass as bass
    import concourse.mybir as mybir
    from concourse import bacc
    from concourse.tile import TileContext

    fp32 = mybir.dt.float32
    fp16 = mybir.dt.float16
    AF = mybir.ActivationFunctionType
    ALU = mybir.AluOpType

    nc = bacc.Bacc("TRN2", target_bir_lowering=False, debug=False)

    xp_d = nc.dram_tensor("xp", [CI_T, P, XCOLS], fp16, kind="ExternalInput")
    lat_d = nc.dram_tensor("lat", [LAT], fp16, kind="ExternalInput")
    wt_d = nc.dram_tensor("wt", [CI_T, P, CO_T * 18 * P], fp16, kind="ExternalInput")
    fct_d = nc.dram_tensor("fct", [LA_T, P, CIN], fp16, kind="ExternalInput")
    ssq_d = nc.dram_tensor("ssq", [CI_T, P, COUT], fp16, kind="ExternalInput")
    fcb_d = nc.dram_tensor("fcb", [P, CI_T], fp32, kind="ExternalInput")
    cb_d = nc.dram_tensor("cbias", [P, CO_T], fp32, kind="ExternalInput")
    out_d = nc.dram_tensor("out", [COUT, H, W], fp32, kind="ExternalOutput")

    inv_wmc2 = 1.0 / (W_MUL_CONV * W_MUL_CONV)

    with TileContext(nc) as tc, tc.tile_pool(name="persist", bufs=1) as persist:
        def tile0(shape, dtype, name):
            return persist.tile(shape, dtype, tag=name, name=name)

        # --- constants / weights: DMA'd once ---
        fct = [tile0([P, CIN], fp16, f"fct{i}") for i in range(LA_T)]
        fcb = tile0([P, CI_T], fp32, "fcb")
        cb = tile0([P, CO_T], fp32, "cb")
        ssq = [tile0([P, COUT], fp16, f"ssq{i}") for i in range(CI_T)]
        wsb = [tile0([P, CO_T * 18 * P], fp16, f"wsb{i}") for i in range(CI_T)]

        for l in range(LA_T):
            nc.sync.dma_start(fct[l][:], fct_d[l])
        nc.sync.dma_start(fcb[:], fcb_d[:])
        nc.sync.dma_start(cb[:], cb_d[:])
        for ci in range(CI_T):
            nc.sync.dma_start(ssq[ci][:], ssq_d[ci])

        with (
            tc.tile_pool(name="xpool", bufs=1) as xpool,
            tc.tile_pool(name="vpool", bufs=1) as vpool,
            tc.tile_pool(name="vtmp", bufs=1) as vtpool,
            tc.tile_pool(name="mpool", bufs=1) as mpool,
            tc.tile_pool(name="spool", bufs=2) as spool,
            tc.tile_pool(name="pconv", bufs=8, space="PSUM") as pconv,
            tc.tile_pool(name="ypool", bufs=1) as ypool,
            tc.tile_pool(name="ycpool", bufs=2) as ycpool,
            tc.tile_pool(name="ytout", bufs=2) as ytpool,
            tc.tile_pool(name="dtmp", bufs=1) as dpool,
        ):
            state = {}  # per-rep tiles: xmod, dscale, V halves

            def emit_inputs_style_demod(rep):
                """latsb/xmod DMA + style matvec + modulate + demod for `rep`."""
                latsb = spool.tile([P, LA_T], fp16, tag="latsb", name=f"latsb_{rep}")
                nc.sync.dma_start(latsb[:], lat_d[:].rearrange("(l p) -> p l", p=P))
                xmod = [
                    xpool.tile([P, XCOLS], fp16, tag=f"xmod{i}", name=f"xmod{i}_{rep}")
                    for i in range(CI_T)
                ]
                for ci in range(CI_T):
                    nc.sync.dma_start(xmod[ci][:], xp_d[ci])
                style = [
                    spool.tile([P, 1], fp32, tag=f"style{i}", name=f"style{i}_{rep}")
                    for i in range(CI_T)
                ]
                style2 = [
                    spool.tile([P, 1], fp16, tag=f"style2_{i}", name=f"style2_{i}_{rep}")
                    for i in range(CI_T)
                ]
                dscale = [
                    spool.tile([P, 1], fp32, tag=f"dscale{i}", name=f"dscale{i}_{rep}")
                    for i in range(CO_T)
                ]
                for ci in range(CI_T):
                    ps = pconv.tile(
                        [P, NMAX], fp32, tag="ps_conv", name=f"ps_st{ci}_{rep}"
                    )[:, :1]
                    for l in range(LA_T):
                        nc.tensor.matmul(
                            ps[:],
                            lhsT=fct[l][:, ci * P : (ci + 1) * P],
                            rhs=latsb[:, l : l + 1],
                            start=(l == 0),
                            stop=(l == LA_T - 1),
                        )
                    nc.scalar.activation(
                        style[ci][:], ps[:], AF.Identity,
                        bias=fcb[:, ci : ci + 1], scale=W_MUL_FC,
                    )
                    nc.scalar.activation(
                        style2[ci][:], ps[:], AF.Square,
                        bias=fcb[:, ci : ci + 1], scale=W_MUL_FC,
                    )
                    nc.vector.tensor_scalar_mul(xmod[ci][:], xmod[ci][:], style[ci][:])

                # demod scale d[o] (matvec + sqrt + recip + 1 Newton step)
                for co in range(CO_T):
                    ps = pconv.tile(
                        [P, NMAX], fp32, tag="ps_conv", name=f"ps_d{co}_{rep}"
                    )[:, :1]
                    for ci in range(CI_T):
                        nc.tensor.matmul(
                            ps[:],
                            lhsT=ssq[ci][:, co * P : (co + 1) * P],
                            rhs=style2[ci][:],
                            start=(ci == 0),
                            stop=(ci == CI_T - 1),
                        )
                    sarg = dpool.tile([P, 1], fp32, tag="sarg", name=f"sarg{co}_{rep}")
                    sq = dpool.tile([P, 1], fp32, tag="sq", name=f"sq{co}_{rep}")
                    y0 = dpool.tile([P, 1], fp32, tag="y0", name=f"y0_{co}_{rep}")
                    u = dpool.tile([P, 1], fp32, tag="u", name=f"u{co}_{rep}")
                    v = dpool.tile([P, 1], fp32, tag="v", name=f"v{co}_{rep}")
                    # sarg = s / wmc^2 ; target d = 1/sqrt(sarg). EPS=1e-8 vs
                    # s ~ O(1e3) is ~1e-11 relative - dropped (no const-AP).
                    nc.scalar.activation(
                        sarg[:], ps[:], AF.Identity, bias=0.0, scale=inv_wmc2
                    )
                    nc.scalar.activation(sq[:], ps[:], AF.Sqrt, bias=0.0, scale=inv_wmc2)
                    nc.vector.reciprocal(y0[:], sq[:])
                    # Newton: y1 = y0*(1.5 - 0.5*sarg*y0^2) - ACT sqrt is low-ULP
                    nc.vector.tensor_mul(u[:], y0[:], y0[:])
                    nc.vector.tensor_mul(v[:], u[:], sarg[:])
                    nc.vector.tensor_scalar(
                        v[:], v[:], -0.5, 1.5, op0=ALU.mult, op1=ALU.add
                    )
                    nc.vector.tensor_mul(dscale[co][:], y0[:], v[:])
                state[rep] = {"xmod": xmod, "dscale": dscale}

            def emit_vbuild_half(rep, h):
                """B^T width combos for row half h (rows 32h .. 32h+33)."""
                xmod = state[rep]["xmod"]
                Vhh = [
                    [
                        vpool.tile(
                            [P, VHC], fp16, tag=f"V{h}_{g}_{ci}",
                            name=f"V{h}_{g}_{ci}_{rep}",
                        )
                        for ci in range(CI_T)
                    ]
                    for g in range(6)
                ]
                state[rep][f"V{h}"] = Vhh
                r0 = 32 * h  # 0 or 32; covers HR=34 rows
                for ci in range(CI_T):
                    def Pl(a):
                        return xmod[ci][
                            :, G + a * PLC : G + (a + 1) * PLC
                        ].rearrange("p (h j) -> p h j", j=JW)[:, r0 : r0 + HR, :]

                    d0 = Pl(0)[:, :, 0:NJ]
                    d1 = Pl(1)[:, :, 0:NJ]
                    d2 = Pl(2)[:, :, 0:NJ]
                    d3 = Pl(3)[:, :, 0:NJ]
                    d4 = Pl(0)[:, :, 1 : NJ + 1]
                    d5 = Pl(1)[:, :, 1 : NJ + 1]

                    def vt(g):
                        return Vhh[g][ci][:].rearrange("p (h j) -> p h j", j=NJ)

                    def tmp(nm):
                        t = vtpool.tile(
                            [P, VHC], fp16, tag=nm, name=f"{nm}{h}_{ci}_{rep}"
                        )
                        return t[:].rearrange("p (h j) -> p h j", j=NJ)

                    # g0=4(d0-d2)+(d4-d2), g1=(d3+d4)-4(d1+d2),
                    # g2=4(d1-d2)+(d4-d3), g3/g4=(d4-d2)+-2(d3-d1),
                    # g5=4(d1-d3)+(d5-d3); emitted in g order so the conv's
                    # g0 matmuls start before later planes land.
                    a2 = tmp("ca")
                    t1 = tmp("cb")
                    nc.vector.tensor_sub(a2, d4, d2)
                    nc.vector.tensor_sub(t1, d0, d2)
                    nc.vector.tensor_scalar_mul(t1, t1, 4.0)
                    nc.vector.tensor_add(vt(0), t1, a2)
                    b2 = tmp("cc")
                    b1 = tmp("cb")
                    nc.vector.tensor_add(b2, d1, d2)
                    nc.vector.tensor_scalar_mul(b2, b2, 4.0)
                    nc.vector.tensor_add(b1, d3, d4)
                    nc.vector.tensor_sub(vt(1), b1, b2)
                    c2 = tmp("cb")
                    c1x = tmp("cc")
                    nc.vector.tensor_sub(c2, d1, d2)
                    nc.vector.tensor_scalar_mul(c2, c2, 4.0)
                    nc.vector.tensor_sub(c1x, d4, d3)
                    nc.vector.tensor_add(vt(2), c2, c1x)
                    bb = tmp("cb")
                    nc.vector.tensor_sub(bb, d3, d1)
                    nc.vector.tensor_scalar_mul(bb, bb, 2.0)
                    nc.vector.tensor_add(vt(3), a2, bb)
                    nc.vector.tensor_sub(vt(4), a2, bb)
                    e2 = tmp("cc")
                    nc.vector.tensor_sub(e2, d5, d3)
                    # V5 = -2*bb + e2 = 4(d1-d3) + (d5-d3)
                    nc.vector.scalar_tensor_tensor(
                        vt(5), bb, -2.0, e2, op0=ALU.mult, op1=ALU.add
                    )

            prev_blk = [None]

            def flush_block(pb):
                pco, pbb, pyc, pyt = pb
                pytv = pyt[:].rearrange("p (r j t) -> p r j t", j=NJ, t=4)
                for t in range(4):
                    nc.scalar.activation(
                        pytv[:, :, :, t].rearrange("p r j -> p (r j)"),
                        pyc[t][:], AF.Prelu,
                        bias=cb[:, pco : pco + 1], scale=1.0, alpha=0.2,
                    )
                nc.sync.dma_start(
                    out_d[pco * P : (pco + 1) * P, 32 * pbb : 32 * pbb + 32, :],
                    pyt[:].rearrange("p (r w) -> p r w", w=W),
                )

            def emit_conv_half(rep, b):
                """All 4 co-tiles for row-block b, reading V half b."""
                dscale = state[rep]["dscale"]
                Vhb = state[rep][f"V{b}"]
                for co in range(CO_T):
                    msb = []
                    for g in range(6):
                        ps = pconv.tile(
                            [P, NMAX], fp32, tag="ps_conv", name=f"pc{co}_{b}_{g}_{rep}"
                        )
                        idx = 0
                        # ci-outer: the group's first matmuls need only
                        # V[g][0], so PE starts before later transforms land
                        for ci in range(CI_T):
                            for kh in range(3):
                                off = kh * NJ
                                nc.tensor.matmul(
                                    ps[:],
                                    lhsT=wsb[ci][
                                        :,
                                        ((co * 3 + kh) * 6 + g) * P : ((co * 3 + kh) * 6 + g + 1) * P,
                                    ],
                                    rhs=Vhb[g][ci][:, off : off + NMAX],
                                    start=(idx == 0),
                                    stop=(idx == 11),
                                )
                                idx += 1
                        m = mpool.tile(
                            [P, NMAX], fp16, tag=f"m{g}", name=f"m{g}_{co}_{b}_{rep}"
                        )
                        # demod scale folds in here (distributes over the sum)
                        nc.scalar.activation(
                            m[:], ps[:], AF.Identity, bias=0.0, scale=dscale[co][:]
                        )
                        msb.append(m)

                    # previous block's activations + store emit here, AFTER
                    # this block's evacs: the in-order ACT queue then frees
                    # PSUM banks without waiting on the DVE combines
                    if prev_blk[0] is not None:
                        flush_block(prev_blk[0])
                        prev_blk[0] = None

                    def ytile(nm):
                        return ypool.tile(
                            [P, NMAX], fp16, tag=nm, name=f"{nm}_{co}_{b}_{rep}"
                        )

                    tp = ytile("tp")
                    tq = ytile("tq")
                    tr = ytile("tr")
                    ts = ytile("ts")
                    yc = [
                        ycpool.tile(
                            [P, NMAX], fp16, tag=f"yc{t}", name=f"yc{t}_{co}_{b}_{rep}"
                        )
                        for t in range(4)
                    ]
                    yt = ytpool.tile(
                        [P, 4 * NMAX], fp32, tag="yt", name=f"yt_{co}_{b}_{rep}"
                    )

                    nc.vector.tensor_add(tp[:], msb[1][:], msb[2][:])
                    nc.vector.tensor_sub(tq[:], msb[1][:], msb[2][:])
                    nc.vector.tensor_add(tr[:], msb[3][:], msb[4][:])
                    nc.vector.tensor_sub(ts[:], msb[3][:], msb[4][:])
                    nc.vector.tensor_add(yc[0][:], tp[:], msb[0][:])
                    nc.vector.tensor_add(yc[0][:], yc[0][:], tr[:])
                    nc.vector.tensor_scalar_mul(ts[:], ts[:], 2.0)
                    nc.vector.tensor_add(yc[1][:], tq[:], ts[:])
                    nc.vector.tensor_scalar_mul(tr[:], tr[:], 4.0)
                    nc.vector.tensor_add(yc[2][:], tp[:], tr[:])
                    nc.vector.tensor_scalar_mul(ts[:], ts[:], 4.0)
                    nc.vector.tensor_add(ts[:], ts[:], msb[5][:])
                    nc.vector.tensor_add(yc[3][:], tq[:], ts[:])
                    prev_blk[0] = (co, b, yc, yt)

            # --- software-pipelined rep loop ---
            # weights: co=0 chunks first so the first conv matmuls can start
            for ci in range(CI_T):
                nc.sync.dma_start(wsb[ci][:, : 18 * P], wt_d[ci, :, : 18 * P])
            emit_inputs_style_demod(0)
            for co in range(1, CO_T):
                for ci in range(CI_T):
                    s = co * 18 * P
                    nc.sync.dma_start(
                        wsb[ci][:, s : s + 18 * P], wt_d[ci, :, s : s + 18 * P]
                    )
            emit_vbuild_half(0, 0)
            for rep in range(reps):
                emit_vbuild_half(rep, 1)
                emit_conv_half(rep, 0)
                if rep + 1 < reps:
                    emit_inputs_style_demod(rep + 1)
                    emit_vbuild_half(rep + 1, 0)
                emit_conv_half(rep, 1)
            if prev_blk[0] is not None:
                flush_block(prev_blk[0])
                prev_blk[0] = None

    nc.compile()
    return nc


def _get_compiled(reps=1):
    if reps not in _COMPILED:
        _COMPILED[reps] = _build_nc(reps)
    return _COMPILED[reps]


def _prep_inputs(x, latent, weight, bias, fc_weight, fc_bias):
    """Host-side layout preprocessing (no model FLOPs besides ssq reduction)."""
    fp16 = np.float16
    # 4 width-phase planes per sample: plane a holds padded cols 4j+a,
    # j=0..16, rows 0..65 -> [B, CIN, 4, 66, 17]
    xp72 = np.pad(x, ((0, 0), (0, 0), (1, 1), (1, WP - W - 1)))  # [B,CIN,66,72]
    ph = np.stack([xp72[:, :, :, a::4][:, :, :, :JW] for a in range(4)], axis=2)
    xpad = np.zeros((B, CIN, XCOLS), np.float32)
    xpad[:, :, G : G + 4 * PLC] = ph.reshape(B, CIN, 4 * PLC)
    xp = np.ascontiguousarray(xpad.reshape(B, CI_T, P, XCOLS)).astype(fp16)

    # width-Winograd weight transform U = G w (G for F(4,3)), as lhsT tiles:
    # wt[ci, p, ((co*3+kh)*6+g)*P + m] = U_g(weight[co*P+m, ci*P+p, kh, :])
    w6 = weight.astype(np.float64).reshape(CO_T, P, CI_T, P, 3, 3)
    Gm = np.array(
        [
            [1 / 4, 0, 0],
            [-1 / 6, -1 / 6, -1 / 6],
            [-1 / 6, 1 / 6, -1 / 6],
            [1 / 24, 1 / 12, 1 / 6],
            [1 / 24, -1 / 12, 1 / 6],
            [0, 0, 1],
        ]
    )
    U = np.einsum("gw,omipkw->omipkg", Gm, w6)  # [co, m, ci, p, kh, g]
    wt = np.ascontiguousarray(U.transpose(2, 3, 0, 4, 5, 1)).reshape(
        CI_T, P, CO_T * 18 * P
    ).astype(fp16)

    fct = np.ascontiguousarray(fc_weight.T).reshape(LA_T, P, CIN).astype(fp16)
    ssq = np.ascontiguousarray(
        (weight.astype(np.float64) ** 2).sum(axis=(2, 3)).T
    ).reshape(CI_T, P, COUT).astype(fp16)
    fcb = np.ascontiguousarray(fc_bias.reshape(CI_T, P).T).astype(np.float32)
    cb = np.ascontiguousarray(bias.reshape(CO_T, P).T).astype(np.float32)
    lat = np.ascontiguousarray(latent).astype(fp16)

    in_maps = []
    for b in range(B):
        in_maps.append(
            {
                "xp": xp[b],
                "lat": lat[b],
                "wt": wt,
                "fct": fct,
                "ssq": ssq,
                "fcb": fcb,
                "cbias": cb,
            }
        )
    return in_maps


def kernel(x, latent, weight, bias, fc_weight, fc_bias):
    from concourse.bass_utils import run_bass_kernel_spmd

    x = np.asarray(x, np.float32)
    latent = np.asarray(latent, np.float32)
    weight = np.asarray(weight, np.float32)
    bias = np.asarray(bias, np.float32)
    fc_weight = np.asarray(fc_weight, np.float32)
    fc_bias = np.asarray(fc_bias, np.float32)

    nc = _get_compiled()
    in_maps = _prep_inputs(x, latent, weight, bias, fc_weight, fc_bias)
    res = run_bass_kernel_spmd(nc, in_maps, core_ids=list(range(B)))
    out = np.stack([res.results[b]["out"] for b in range(B)], axis=0)
    return out.astype(np.float32)
